# revision 18
# baseline (speedup 1.0000x reference)
"""AgentScaleDotProductAttention on 8 TRN2 NeuronCores.

Sharding: core c handles batch c//2, seq-half c%2 (2048 query/key rows).
Stage 1 (agents attend to k/v over the full 4096 keys) is computed on local
key shards and combined with a pairwise AllReduce of the unnormalized
(softmax numerator @ v, softmax denominator) pair — softmax is computed
without max subtraction (scores are O(+-13), safely inside fp32 exp range).
Stage 2 (queries attend to the 64 agents) is embarrassingly parallel.

Host-side prep feeds each core transposed layouts ([d, s] "T" tensors) so
every matmul on the device contracts along the partition axis with zero
on-chip transposes.
"""

import math
import os
import sys

sys.path.insert(0, "/opt/trn_rl_repo")

import numpy as np

import concourse.bass as bass
import concourse.mybir as mybir
from concourse.bass_utils import run_bass_kernel_spmd
from concourse.tile import TileContext

B = 4
S = 4096
D = 1024
A = 64
HEADS = 16
HEAD_DIM = 64
SCALE = 1.0 / math.sqrt(HEAD_DIM)
N_CORES = 8
S_LOC = S // 2          # 2048 rows per core
JT = D // 128           # 8 chunks of the model dim
ST = S_LOC // 128       # 16 seq tiles of 128
SBLK = S_LOC // 512     # 4 seq blocks of 512
CC_GROUPS = [[0, 1], [2, 3], [4, 5], [6, 7]]

MM_DT = os.environ.get("AGENT_MM_DT", "fp32")  # fp32 | fp32r | bf16

_MAX_WAITS = 1


def _split_multi_waits(nc, max_waits=_MAX_WAITS):
    """This walrus build supports one sync wait per instruction; move the
    surplus waits Tile emits (e.g. on its final drain) onto same-engine nops
    placed immediately before the offender."""
    n = 0
    for f in nc.m.functions:
        for bb in f.blocks:
            out = []
            for ins in bb.instructions:
                if "PoolBoundary" in type(ins).__name__:
                    # Non-executable allocator marker; Tile's lowering skips
                    # these but a never-freed single-tile pool's release can
                    # leak through, and walrus aborts on the unknown opcode.
                    continue
                si = ins.sync_info
                if si is not None and si.on_wait and len(si.on_wait) > max_waits:
                    waits = list(si.on_wait)
                    extra, keep = waits[:-max_waits], waits[-max_waits:]
                    for i in range(0, len(extra), max_waits):
                        chunk = extra[i : i + max_waits]
                        nop = mybir.InstNoOp(
                            name=f"{ins.name}-ws{i}",
                            engine=ins.engine,
                            sync_info=mybir.SyncInfo(on_wait=chunk, on_update=[]),
                            bass_nofuse=True,
                        )
                        out.append(nop)
                        n += 1
                    si.on_wait = keep
                out.append(ins)
            # NB: attribute assignment — bb.instructions returns a copy, so
            # in-place slice assignment would be silently dropped.
            bb.instructions = out
    return n


def build_nc(mm_dt=MM_DT):
    f32 = mybir.dt.float32
    if mm_dt == "bf16":
        dt_in = mybir.dt.bfloat16
    elif mm_dt == "fp32r":
        dt_in = mybir.dt.float32r
    else:
        dt_in = f32
    mm = lambda ap: ap  # noqa: E731
    dt_cc = mybir.dt.bfloat16 if mm_dt == "bf16" else f32

    nc = bass.Bass()
    wT = nc.dram_tensor("wT", [D, D], dt_in, kind="ExternalInput")
    qh = nc.dram_tensor("qh", [128, JT * A], dt_in, kind="ExternalInput")
    bias = nc.dram_tensor("bias", [128, JT], f32, kind="ExternalInput")
    kT = nc.dram_tensor("kT", [D, S_LOC], dt_in, kind="ExternalInput")
    vv = nc.dram_tensor("v", [S_LOC, D], dt_in, kind="ExternalInput")
    qT = nc.dram_tensor("qT", [D, S_LOC], dt_in, kind="ExternalInput")
    out = nc.dram_tensor("out", [S_LOC, D], f32, kind="ExternalOutput")

    EXP = mybir.ActivationFunctionType.Exp

    with TileContext(nc) as tc:
        cc_prim_in, _ = tc.tile([A, 2], f32, space="DRAM", name="cc_prim_in")
        cc_prim_out, _ = tc.tile(
            [2 * A, 2], f32, space="DRAM", addr_space="Shared", name="cc_prim_out"
        )
        cc_in, _ = tc.tile([A, D + 1], dt_cc, space="DRAM", name="cc_in")
        cc_out, _ = tc.tile(
            [2 * A, D + 1], dt_cc, space="DRAM", addr_space="Shared", name="cc_out"
        )
        with (
            tc.tile_pool(name="sb", bufs=1) as sb,
            tc.tile_pool(name="ps", bufs=1, space="PSUM") as ps,
        ):
            # ---- loads (early, in priority order) ----
            qh_sb = sb.tile([128, JT * A], dt_in, name="qh_sb")
            nc.sync.dma_start(out=qh_sb[:], in_=qh[:])
            bias_sb = sb.tile([128, JT], f32, name="bias_sb")
            nc.sync.dma_start(out=bias_sb[:], in_=bias[:])
            # N=1 matmuls and fp32r memsets are ISA-invalid; keep the ones
            # column 2 wide (even free size) and produce it via f32 memset +
            # copy-cast so the fp32r "producer must round" rule is satisfied.
            ones_f32 = sb.tile([128, 2], f32, name="ones_f32")
            nc.vector.memset(ones_f32[:], 1.0)
            ones_sb = sb.tile([128, 2], dt_in, name="ones_sb")
            nc.vector.tensor_copy(ones_sb[:], ones_f32[:])

            # Tiny primer collective: absorbs the ~15-20us ncfw wake-up
            # latency so the real AllGather starts moving data promptly.
            prim_sb = sb.tile([A, 2], f32, name="prim_sb")
            nc.vector.memset(prim_sb[:], 0.0)
            nc.gpsimd.dma_start(out=cc_prim_in[:], in_=prim_sb[:])
            nc.gpsimd.collective_compute(
                "AllGather",
                mybir.AluOpType.bypass,
                ins=[cc_prim_in[:]],
                outs=[cc_prim_out[:]],
                replica_groups=CC_GROUPS,
            )

            wt_sb = []
            for dt_i in range(JT):
                t = sb.tile([128, D], dt_in, tag="wt", bufs=JT, name=f"wt{dt_i}")
                nc.sync.dma_start(out=t[:], in_=wT[dt_i * 128 : (dt_i + 1) * 128, :])
                wt_sb.append(t)
            kt_sb = []
            for jt in range(JT):
                t = sb.tile([128, S_LOC], dt_in, tag="big", bufs=12, name=f"kt{jt}")
                nc.sync.dma_start(out=t[:], in_=kT[jt * 128 : (jt + 1) * 128, :])
                kt_sb.append(t)

            # ---- phase A: agent tokens  atT[j, a] ----
            atT_sb = sb.tile([128, JT * A], dt_in, name="atT_sb")
            for jt in range(JT):
                at_ps = ps.tile([128, A], mybir.dt.float32, tag="sc", bufs=2,
                                name=f"at_ps{jt}")
                for dt_i in range(JT):
                    nc.tensor.matmul(
                        at_ps[:],
                        mm(wt_sb[dt_i][:, jt * 128 : (jt + 1) * 128]),
                        mm(qh_sb[:, dt_i * A : (dt_i + 1) * A]),
                        start=(dt_i == 0),
                        stop=(dt_i == JT - 1),
                    )
                nc.vector.tensor_scalar_add(
                    atT_sb[:, jt * A : (jt + 1) * A], at_ps[:], bias_sb[:, jt : jt + 1]
                )

            # ---- phase B: stage-1 scores + weighted values (local shard) ----
            av_ps = ps.tile([A, D], mybir.dt.float32, tag="avo", bufs=2,
                            name="av_ps")
            den1_ps = ps.tile([A, 2], mybir.dt.float32, tag="den", bufs=2,
                              name="den1_ps")
            # Software-pipelined one step: st's score matmuls are emitted
            # before (st-1)'s p1@v matmuls so the PE never waits on the exp.
            v_ts, p1_ts = {}, {}
            for st in range(ST + 1):
                if st < ST:
                    v_t = sb.tile([128, D], dt_in, tag="v", bufs=4, name=f"v{st}")
                    nc.sync.dma_start(
                        out=v_t[:], in_=vv[st * 128 : (st + 1) * 128, :]
                    )
                    v_ts[st] = v_t
                    s1_ps = ps.tile([128, A], mybir.dt.float32, tag="sc", bufs=2,
                                    name=f"s1_ps{st}")
                    for jt in range(JT):
                        nc.tensor.matmul(
                            s1_ps[:],
                            mm(kt_sb[jt][:, st * 128 : (st + 1) * 128]),
                            mm(atT_sb[:, jt * A : (jt + 1) * A]),
                            start=(jt == 0),
                            stop=(jt == JT - 1),
                        )
                    p1_t = sb.tile([128, A], dt_in, tag="p1", bufs=4, name=f"p1{st}")
                    nc.scalar.activation(p1_t[:], s1_ps[:], EXP, scale=SCALE)
                    p1_ts[st] = p1_t
                if st >= 1:
                    pv, p1p = v_ts.pop(st - 1), p1_ts[st - 1]
                    first, last = st - 1 == 0, st - 1 == ST - 1
                    nc.tensor.matmul(av_ps[:, 0:512], mm(p1p[:]), mm(pv[:, 0:512]),
                                     start=first, stop=last, skip_group_check=True)
                    nc.tensor.matmul(av_ps[:, 512:1024], mm(p1p[:]),
                                     mm(pv[:, 512:1024]),
                                     start=first, stop=last, skip_group_check=True)
                    nc.tensor.matmul(den1_ps[:], mm(p1p[:]), mm(ones_sb[:]),
                                     start=first, stop=last, skip_group_check=True)
            avun_sb = sb.tile([A, D + 1], dt_cc, name="avun_sb")
            avun_copy = nc.vector.tensor_copy(avun_sb[:, 0:D], av_ps[:])
            nc.vector.tensor_copy(avun_sb[:, D : D + 1], den1_ps[:, 0:1])
            nc.gpsimd.dma_start(out=cc_in[:], in_=avun_sb[:])
            nc.gpsimd.collective_compute(
                "AllGather",
                mybir.AluOpType.bypass,
                ins=[cc_in[:]],
                outs=[cc_out[:]],
                replica_groups=CC_GROUPS,
            )

            # qt loads are emitted only now: the sync-DGE queues are FIFO,
            # so emitting them earlier would make the v strips (needed per
            # iteration above) queue behind 4 MiB of prefetch.
            qt_sb = []
            for jt in range(JT):
                t = sb.tile([128, S_LOC], dt_in, tag="big", bufs=12, name=f"qt{jt}")
                nc.sync.dma_start(out=t[:], in_=qT[jt * 128 : (jt + 1) * 128, :])
                qt_sb.append(t)

            # ---- phase D: stage-2 scores  s2T[a, s]  (overlaps the AG) ----
            from concourse.tile import add_dep_helper

            p2_sb = sb.tile([A, S_LOC], dt_in, name="p2_sb")
            for sb_i in range(SBLK):
                s2_ps = ps.tile([A, 512], mybir.dt.float32, tag="sc", bufs=2,
                                name=f"s2_ps{sb_i}")
                for jt in range(JT):
                    mm_i = nc.tensor.matmul(
                        s2_ps[:],
                        mm(atT_sb[:, jt * A : (jt + 1) * A]),
                        mm(qt_sb[jt][:, sb_i * 512 : (sb_i + 1) * 512]),
                        start=(jt == 0),
                        stop=(jt == JT - 1),
                    )
                    if sb_i == 0 and jt == 0:
                        # keep phase D out of phase B's PE gaps: it is the
                        # only work that can cover the AllGather latency
                        add_dep_helper(mm_i.ins, avun_copy.ins, sync=True,
                                       reason="phase D covers the AllGather")
                nc.scalar.activation(
                    p2_sb[:, sb_i * 512 : (sb_i + 1) * 512], s2_ps[:], EXP, scale=SCALE
                )

            # stage-2 denominators (independent of the AR; runs under it)
            rec2_sb = sb.tile([128, ST], f32, name="rec2_sb")
            for st in range(ST):
                sl = slice(st * 128, (st + 1) * 128)
                den2_ps = ps.tile([128, 2], mybir.dt.float32, tag="den", bufs=2,
                                  name=f"den2_{st}")
                nc.tensor.matmul(den2_ps[:], mm(p2_sb[:, sl]), mm(ones_sb[:A, :]),
                                 start=True, stop=True)
                nc.vector.reciprocal(rec2_sb[:, st : st + 1], den2_ps[:, 0:1])

            # combine the gathered rank slices, normalize
            avg_sb = []
            for r in range(2):
                t = sb.tile([A, D + 1], dt_cc, tag="avg", bufs=2,
                            name=f"avg_sb{r}")
                nc.gpsimd.dma_start(
                    out=t[:], in_=cc_out[r * A : (r + 1) * A, :]
                )
                avg_sb.append(t)
            avt_sb = sb.tile([A, D + 1], f32, name="avt_sb")
            nc.vector.tensor_add(avt_sb[:], avg_sb[0][:], avg_sb[1][:])
            rec1 = sb.tile([A, 1], f32, name="rec1")
            nc.vector.reciprocal(rec1[:], avt_sb[:, D : D + 1])
            av_sb = sb.tile([A, D], dt_in, name="av_sb")
            nc.vector.tensor_scalar_mul(av_sb[:], avt_sb[:, 0:D], rec1[:, 0:1])

            # ---- phase E: out[s, j] = softmax(s2) @ av ----
            # Each st's two half-tiles use different PSUM tags (4 banks in
            # rotation) so the PE can run ahead of the scaling ops; halves
            # are scaled on DVE and ScalarE concurrently and DMA'd out
            # independently for finer pipelining.
            for st in range(ST):
                sl = slice(st * 128, (st + 1) * 128)
                out_t = sb.tile([128, D], f32, tag="outs", bufs=3, name=f"out{st}")
                for nb in range(2):
                    o_ps = ps.tile(
                        [128, 512], mybir.dt.float32,
                        tag=("sc" if nb == 0 else "avo"), bufs=2,
                        name=f"o{st}_{nb}",
                    )
                    nc.tensor.matmul(
                        o_ps[:],
                        mm(p2_sb[:, sl]),
                        mm(av_sb[:, nb * 512 : (nb + 1) * 512]),
                        start=True,
                        stop=True,
                    )
                    if nb == 0:
                        nc.vector.tensor_scalar_mul(
                            out_t[:, 0:512], o_ps[:], rec2_sb[:, st : st + 1]
                        )
                    else:
                        nc.scalar.activation(
                            out_t[:, 512:1024], o_ps[:],
                            mybir.ActivationFunctionType.Copy,
                            scale=rec2_sb[:, st : st + 1],
                        )
                    nc.sync.dma_start(
                        out=out[sl, nb * 512 : (nb + 1) * 512],
                        in_=out_t[:, nb * 512 : (nb + 1) * 512],
                    )

    # The first pass over bb.instructions can trigger a lazy flush that
    # appends deferred instructions after our rewrite; iterate to a clean
    # fixpoint and verify.
    for _ in range(4):
        _split_multi_waits(nc)
        dirty = any(
            "PoolBoundary" in type(i).__name__
            or (i.sync_info is not None and len(i.sync_info.on_wait or []) > _MAX_WAITS)
            for f in nc.m.functions
            for bb in f.blocks
            for i in bb.instructions
        )
        if not dirty:
            break
    assert not dirty, "BIR post-pass did not converge"
    return nc


def prepare_in_maps(q, k, v, agent_w, agent_b, mm_dt=MM_DT):
    if mm_dt == "bf16":
        import ml_dtypes

        dt_h = ml_dtypes.bfloat16
    else:
        dt_h = np.float32
    q = np.asarray(q, dtype=np.float32)
    k = np.asarray(k, dtype=np.float32)
    v = np.asarray(v, dtype=np.float32)
    agent_w = np.asarray(agent_w, dtype=np.float32)
    agent_b = np.asarray(agent_b, dtype=np.float32)

    wT_h = np.ascontiguousarray(agent_w.T).astype(dt_h)
    bias_h = np.ascontiguousarray(agent_b.reshape(JT, 128).T)  # [128, JT] f32
    in_maps = []
    for c in range(N_CORES):
        ib, h = c // 2, c % 2
        sl = slice(h * S_LOC, (h + 1) * S_LOC)
        qh_h = (
            np.ascontiguousarray(q[ib, :A, :].T)  # [D, A]
            .reshape(JT, 128, A)
            .transpose(1, 0, 2)
            .reshape(128, JT * A)
            .astype(dt_h)
        )
        in_maps.append(
            {
                "wT": np.ascontiguousarray(wT_h),
                "qh": np.ascontiguousarray(qh_h),
                "bias": bias_h,
                "kT": np.ascontiguousarray(k[ib, sl, :].T).astype(dt_h),
                "v": np.ascontiguousarray(v[ib, sl, :]).astype(dt_h),
                "qT": np.ascontiguousarray(q[ib, sl, :].T).astype(dt_h),
            }
        )
    return in_maps


def gather(results):
    out = np.empty((B, S, D), dtype=np.float32)
    for c in range(N_CORES):
        ib, h = c // 2, c % 2
        out[ib, h * S_LOC : (h + 1) * S_LOC, :] = results[c]["out"]
    attention = np.ascontiguousarray(
        out.reshape(B, S, HEADS, HEAD_DIM).transpose(0, 2, 1, 3)
    )
    return out, attention


_NC_CACHE = {}


def get_nc(mm_dt=MM_DT):
    if mm_dt not in _NC_CACHE:
        _NC_CACHE[mm_dt] = build_nc(mm_dt)
    return _NC_CACHE[mm_dt]


def kernel(q, k, v, agent_w, agent_b):
    nc = get_nc()
    in_maps = prepare_in_maps(q, k, v, agent_w, agent_b)
    res = run_bass_kernel_spmd(nc, in_maps, list(range(N_CORES)))
    return gather(res.results)


# revision 20
# speedup vs baseline: 1.7687x; 1.7687x over previous
"""AgentScaleDotProductAttention on 8 TRN2 NeuronCores.

Sharding: core c handles batch c//2, seq-half c%2 (2048 query/key rows).
Stage 1 (agents attend to k/v over the full 4096 keys) is computed on local
key shards and combined with a pairwise AllReduce of the unnormalized
(softmax numerator @ v, softmax denominator) pair — softmax is computed
without max subtraction (scores are O(+-13), safely inside fp32 exp range).
Stage 2 (queries attend to the 64 agents) is embarrassingly parallel.

Host-side prep feeds each core transposed layouts ([d, s] "T" tensors) so
every matmul on the device contracts along the partition axis with zero
on-chip transposes.
"""

import math
import os
import sys

sys.path.insert(0, "/opt/trn_rl_repo")

import numpy as np

import concourse.bass as bass
import concourse.mybir as mybir
from concourse.bass_utils import run_bass_kernel_spmd
from concourse.tile import TileContext

B = 4
S = 4096
D = 1024
A = 64
HEADS = 16
HEAD_DIM = 64
SCALE = 1.0 / math.sqrt(HEAD_DIM)
N_CORES = 8
S_LOC = S // 2          # 2048 rows per core
JT = D // 128           # 8 chunks of the model dim
ST = S_LOC // 128       # 16 seq tiles of 128
SBLK = S_LOC // 512     # 4 seq blocks of 512
CC_GROUPS = [[0, 1], [2, 3], [4, 5], [6, 7]]

MM_DT = os.environ.get("AGENT_MM_DT", "bf16")  # fp32 | fp32r | bf16
OUT_BF16 = os.environ.get("AGENT_OUT_BF16", "0") == "1"

_MAX_WAITS = 1


def _split_multi_waits(nc, max_waits=_MAX_WAITS):
    """This walrus build supports one sync wait per instruction; move the
    surplus waits Tile emits (e.g. on its final drain) onto same-engine nops
    placed immediately before the offender."""
    n = 0
    for f in nc.m.functions:
        for bb in f.blocks:
            out = []
            for ins in bb.instructions:
                if "PoolBoundary" in type(ins).__name__:
                    # Non-executable allocator marker; Tile's lowering skips
                    # these but a never-freed single-tile pool's release can
                    # leak through, and walrus aborts on the unknown opcode.
                    continue
                si = ins.sync_info
                if si is not None and si.on_wait and len(si.on_wait) > max_waits:
                    waits = list(si.on_wait)
                    extra, keep = waits[:-max_waits], waits[-max_waits:]
                    for i in range(0, len(extra), max_waits):
                        chunk = extra[i : i + max_waits]
                        nop = mybir.InstNoOp(
                            name=f"{ins.name}-ws{i}",
                            engine=ins.engine,
                            sync_info=mybir.SyncInfo(on_wait=chunk, on_update=[]),
                            bass_nofuse=True,
                        )
                        out.append(nop)
                        n += 1
                    si.on_wait = keep
                out.append(ins)
            # NB: attribute assignment — bb.instructions returns a copy, so
            # in-place slice assignment would be silently dropped.
            bb.instructions = out
    return n


def build_nc(mm_dt=MM_DT):
    f32 = mybir.dt.float32
    if mm_dt == "bf16":
        dt_in = mybir.dt.bfloat16
    elif mm_dt == "fp32r":
        dt_in = mybir.dt.float32r
    else:
        dt_in = f32
    mm = lambda ap: ap  # noqa: E731
    dt_cc = mybir.dt.bfloat16 if mm_dt == "bf16" else f32

    nc = bass.Bass()
    wT = nc.dram_tensor("wT", [D, D], dt_in, kind="ExternalInput")
    qh = nc.dram_tensor("qh", [128, JT * A], dt_in, kind="ExternalInput")
    bias = nc.dram_tensor("bias", [128, JT], f32, kind="ExternalInput")
    kT = nc.dram_tensor("kT", [D, S_LOC], dt_in, kind="ExternalInput")
    vv = nc.dram_tensor("v", [S_LOC, D], dt_in, kind="ExternalInput")
    qT = nc.dram_tensor("qT", [D, S_LOC], dt_in, kind="ExternalInput")
    dt_out = mybir.dt.bfloat16 if (OUT_BF16 and mm_dt == "bf16") else f32
    out = nc.dram_tensor("out", [S_LOC, D], dt_out, kind="ExternalOutput")

    EXP = mybir.ActivationFunctionType.Exp

    with TileContext(nc) as tc:
        cc_prim_in, _ = tc.tile([A, 2], f32, space="DRAM", name="cc_prim_in")
        cc_prim_out, _ = tc.tile(
            [2 * A, 2], f32, space="DRAM", addr_space="Shared", name="cc_prim_out"
        )
        cc_in, _ = tc.tile([A, D + 1], dt_cc, space="DRAM", name="cc_in")
        cc_out, _ = tc.tile(
            [2 * A, D + 1], dt_cc, space="DRAM", addr_space="Shared", name="cc_out"
        )
        with (
            tc.tile_pool(name="sb", bufs=1) as sb,
            tc.tile_pool(name="ps", bufs=1, space="PSUM") as ps,
        ):
            # ---- loads (early, in priority order) ----
            qh_sb = sb.tile([128, JT * A], dt_in, name="qh_sb")
            nc.sync.dma_start(out=qh_sb[:], in_=qh[:])
            bias_sb = sb.tile([128, JT], f32, name="bias_sb")
            nc.sync.dma_start(out=bias_sb[:], in_=bias[:])
            # N=1 matmuls and fp32r memsets are ISA-invalid; keep the ones
            # column 2 wide (even free size) and produce it via f32 memset +
            # copy-cast so the fp32r "producer must round" rule is satisfied.
            ones_f32 = sb.tile([128, 2], f32, name="ones_f32")
            nc.vector.memset(ones_f32[:], 1.0)
            ones_sb = sb.tile([128, 2], dt_in, name="ones_sb")
            nc.vector.tensor_copy(ones_sb[:], ones_f32[:])

            # Tiny primer collective: absorbs the ~15-20us ncfw wake-up
            # latency so the real AllGather starts moving data promptly.
            prim_sb = sb.tile([A, 2], f32, name="prim_sb")
            nc.vector.memset(prim_sb[:], 0.0)
            nc.gpsimd.dma_start(out=cc_prim_in[:], in_=prim_sb[:])
            nc.gpsimd.collective_compute(
                "AllGather",
                mybir.AluOpType.bypass,
                ins=[cc_prim_in[:]],
                outs=[cc_prim_out[:]],
                replica_groups=CC_GROUPS,
            )

            wt_sb = []
            for dt_i in range(JT):
                t = sb.tile([128, D], dt_in, tag="wt", bufs=JT, name=f"wt{dt_i}")
                nc.sync.dma_start(out=t[:], in_=wT[dt_i * 128 : (dt_i + 1) * 128, :])
                wt_sb.append(t)
            kt_sb = []
            for jt in range(JT):
                t = sb.tile([128, S_LOC], dt_in, tag="big", bufs=12, name=f"kt{jt}")
                nc.sync.dma_start(out=t[:], in_=kT[jt * 128 : (jt + 1) * 128, :])
                kt_sb.append(t)

            # ---- phase A: agent tokens  atT[j, a] ----
            atT_sb = sb.tile([128, JT * A], dt_in, name="atT_sb")
            for jt in range(JT):
                at_ps = ps.tile([128, A], mybir.dt.float32, tag="sc", bufs=2,
                                name=f"at_ps{jt}")
                for dt_i in range(JT):
                    nc.tensor.matmul(
                        at_ps[:],
                        mm(wt_sb[dt_i][:, jt * 128 : (jt + 1) * 128]),
                        mm(qh_sb[:, dt_i * A : (dt_i + 1) * A]),
                        start=(dt_i == 0),
                        stop=(dt_i == JT - 1),
                    )
                nc.vector.tensor_scalar_add(
                    atT_sb[:, jt * A : (jt + 1) * A], at_ps[:], bias_sb[:, jt : jt + 1]
                )

            # ---- phase B: stage-1 scores + weighted values (local shard) ----
            av_ps = ps.tile([A, D], mybir.dt.float32, tag="avo", bufs=2,
                            name="av_ps")
            den1_ps = ps.tile([A, 2], mybir.dt.float32, tag="den", bufs=2,
                              name="den1_ps")
            # Software-pipelined one step: st's score matmuls are emitted
            # before (st-1)'s p1@v matmuls so the PE never waits on the exp.
            v_ts, p1_ts = {}, {}
            for st in range(ST + 1):
                if st < ST:
                    v_t = sb.tile([128, D], dt_in, tag="v", bufs=4, name=f"v{st}")
                    nc.sync.dma_start(
                        out=v_t[:], in_=vv[st * 128 : (st + 1) * 128, :]
                    )
                    v_ts[st] = v_t
                    s1_ps = ps.tile([128, A], mybir.dt.float32, tag="sc", bufs=2,
                                    name=f"s1_ps{st}")
                    for jt in range(JT):
                        nc.tensor.matmul(
                            s1_ps[:],
                            mm(kt_sb[jt][:, st * 128 : (st + 1) * 128]),
                            mm(atT_sb[:, jt * A : (jt + 1) * A]),
                            start=(jt == 0),
                            stop=(jt == JT - 1),
                        )
                    p1_t = sb.tile([128, A], dt_in, tag="p1", bufs=4, name=f"p1{st}")
                    nc.scalar.activation(p1_t[:], s1_ps[:], EXP, scale=SCALE)
                    p1_ts[st] = p1_t
                if st >= 1:
                    pv, p1p = v_ts.pop(st - 1), p1_ts[st - 1]
                    first, last = st - 1 == 0, st - 1 == ST - 1
                    nc.tensor.matmul(av_ps[:, 0:512], mm(p1p[:]), mm(pv[:, 0:512]),
                                     start=first, stop=last, skip_group_check=True)
                    nc.tensor.matmul(av_ps[:, 512:1024], mm(p1p[:]),
                                     mm(pv[:, 512:1024]),
                                     start=first, stop=last, skip_group_check=True)
                    nc.tensor.matmul(den1_ps[:], mm(p1p[:]), mm(ones_sb[:]),
                                     start=first, stop=last, skip_group_check=True)
            avun_sb = sb.tile([A, D + 1], dt_cc, name="avun_sb")
            avun_copy = nc.vector.tensor_copy(avun_sb[:, 0:D], av_ps[:])
            nc.vector.tensor_copy(avun_sb[:, D : D + 1], den1_ps[:, 0:1])
            nc.gpsimd.dma_start(out=cc_in[:], in_=avun_sb[:])
            nc.gpsimd.collective_compute(
                "AllGather",
                mybir.AluOpType.bypass,
                ins=[cc_in[:]],
                outs=[cc_out[:]],
                replica_groups=CC_GROUPS,
            )

            # qt loads are emitted only now: the sync-DGE queues are FIFO,
            # so emitting them earlier would make the v strips (needed per
            # iteration above) queue behind 4 MiB of prefetch.
            qt_sb = []
            for jt in range(JT):
                t = sb.tile([128, S_LOC], dt_in, tag="big", bufs=12, name=f"qt{jt}")
                nc.sync.dma_start(out=t[:], in_=qT[jt * 128 : (jt + 1) * 128, :])
                qt_sb.append(t)

            # ---- phase D: stage-2 scores  s2T[a, s]  (overlaps the AG) ----
            from concourse.tile import add_dep_helper

            p2_sb = sb.tile([A, S_LOC], dt_in, name="p2_sb")
            for sb_i in range(SBLK):
                s2_ps = ps.tile([A, 512], mybir.dt.float32, tag="sc", bufs=2,
                                name=f"s2_ps{sb_i}")
                for jt in range(JT):
                    mm_i = nc.tensor.matmul(
                        s2_ps[:],
                        mm(atT_sb[:, jt * A : (jt + 1) * A]),
                        mm(qt_sb[jt][:, sb_i * 512 : (sb_i + 1) * 512]),
                        start=(jt == 0),
                        stop=(jt == JT - 1),
                    )
                    if sb_i == 0 and jt == 0:
                        # keep phase D out of phase B's PE gaps: it is the
                        # only work that can cover the AllGather latency
                        add_dep_helper(mm_i.ins, avun_copy.ins, sync=True,
                                       reason="phase D covers the AllGather")
                nc.scalar.activation(
                    p2_sb[:, sb_i * 512 : (sb_i + 1) * 512], s2_ps[:], EXP, scale=SCALE
                )

            # stage-2 denominators (independent of the AR; runs under it)
            rec2_sb = sb.tile([128, ST], f32, name="rec2_sb")
            for st in range(ST):
                sl = slice(st * 128, (st + 1) * 128)
                den2_ps = ps.tile([128, 2], mybir.dt.float32, tag="den", bufs=2,
                                  name=f"den2_{st}")
                nc.tensor.matmul(den2_ps[:], mm(p2_sb[:, sl]), mm(ones_sb[:A, :]),
                                 start=True, stop=True)
                nc.vector.reciprocal(rec2_sb[:, st : st + 1], den2_ps[:, 0:1])

            # combine the gathered rank slices, normalize
            avg_sb = []
            for r in range(2):
                t = sb.tile([A, D + 1], dt_cc, tag="avg", bufs=2,
                            name=f"avg_sb{r}")
                nc.gpsimd.dma_start(
                    out=t[:], in_=cc_out[r * A : (r + 1) * A, :]
                )
                avg_sb.append(t)
            avt_sb = sb.tile([A, D + 1], f32, name="avt_sb")
            nc.vector.tensor_add(avt_sb[:], avg_sb[0][:], avg_sb[1][:])
            rec1 = sb.tile([A, 1], f32, name="rec1")
            nc.vector.reciprocal(rec1[:], avt_sb[:, D : D + 1])
            av_sb = sb.tile([A, D], dt_in, name="av_sb")
            nc.vector.tensor_scalar_mul(av_sb[:], avt_sb[:, 0:D], rec1[:, 0:1])

            # ---- phase E: out[s, j] = softmax(s2) @ av ----
            # Each st's two half-tiles use different PSUM tags (4 banks in
            # rotation) so the PE can run ahead of the scaling ops; halves
            # are scaled on DVE and ScalarE concurrently and DMA'd out
            # independently for finer pipelining.
            for st in range(ST):
                sl = slice(st * 128, (st + 1) * 128)
                out_t = sb.tile([128, D], dt_out, tag="outs", bufs=3, name=f"out{st}")
                for nb in range(2):
                    o_ps = ps.tile(
                        [128, 512], mybir.dt.float32,
                        tag=("sc" if nb == 0 else "avo"), bufs=2,
                        name=f"o{st}_{nb}",
                    )
                    nc.tensor.matmul(
                        o_ps[:],
                        mm(p2_sb[:, sl]),
                        mm(av_sb[:, nb * 512 : (nb + 1) * 512]),
                        start=True,
                        stop=True,
                    )
                    if nb == 0:
                        nc.vector.tensor_scalar_mul(
                            out_t[:, 0:512], o_ps[:], rec2_sb[:, st : st + 1]
                        )
                    else:
                        nc.scalar.activation(
                            out_t[:, 512:1024], o_ps[:],
                            mybir.ActivationFunctionType.Copy,
                            scale=rec2_sb[:, st : st + 1],
                        )
                    nc.sync.dma_start(
                        out=out[sl, nb * 512 : (nb + 1) * 512],
                        in_=out_t[:, nb * 512 : (nb + 1) * 512],
                    )

    # The first pass over bb.instructions can trigger a lazy flush that
    # appends deferred instructions after our rewrite; iterate to a clean
    # fixpoint and verify.
    for _ in range(4):
        _split_multi_waits(nc)
        dirty = any(
            "PoolBoundary" in type(i).__name__
            or (i.sync_info is not None and len(i.sync_info.on_wait or []) > _MAX_WAITS)
            for f in nc.m.functions
            for bb in f.blocks
            for i in bb.instructions
        )
        if not dirty:
            break
    assert not dirty, "BIR post-pass did not converge"
    return nc


def prepare_in_maps(q, k, v, agent_w, agent_b, mm_dt=MM_DT):
    if mm_dt == "bf16":
        import ml_dtypes

        dt_h = ml_dtypes.bfloat16
    else:
        dt_h = np.float32
    q = np.asarray(q, dtype=np.float32)
    k = np.asarray(k, dtype=np.float32)
    v = np.asarray(v, dtype=np.float32)
    agent_w = np.asarray(agent_w, dtype=np.float32)
    agent_b = np.asarray(agent_b, dtype=np.float32)

    wT_h = np.ascontiguousarray(agent_w.T).astype(dt_h)
    bias_h = np.ascontiguousarray(agent_b.reshape(JT, 128).T)  # [128, JT] f32
    in_maps = []
    for c in range(N_CORES):
        ib, h = c // 2, c % 2
        sl = slice(h * S_LOC, (h + 1) * S_LOC)
        qh_h = (
            np.ascontiguousarray(q[ib, :A, :].T)  # [D, A]
            .reshape(JT, 128, A)
            .transpose(1, 0, 2)
            .reshape(128, JT * A)
            .astype(dt_h)
        )
        in_maps.append(
            {
                "wT": np.ascontiguousarray(wT_h),
                "qh": np.ascontiguousarray(qh_h),
                "bias": bias_h,
                "kT": np.ascontiguousarray(k[ib, sl, :].T).astype(dt_h),
                "v": np.ascontiguousarray(v[ib, sl, :]).astype(dt_h),
                "qT": np.ascontiguousarray(q[ib, sl, :].T).astype(dt_h),
            }
        )
    return in_maps


def gather(results):
    out = np.empty((B, S, D), dtype=np.float32)
    for c in range(N_CORES):
        ib, h = c // 2, c % 2
        out[ib, h * S_LOC : (h + 1) * S_LOC, :] = np.asarray(
            results[c]["out"], dtype=np.float32
        )
    attention = np.ascontiguousarray(
        out.reshape(B, S, HEADS, HEAD_DIM).transpose(0, 2, 1, 3)
    )
    return out, attention


_NC_CACHE = {}


def get_nc(mm_dt=MM_DT):
    key = (mm_dt, OUT_BF16)
    if key not in _NC_CACHE:
        _NC_CACHE[key] = build_nc(mm_dt)
    return _NC_CACHE[key]


def kernel(q, k, v, agent_w, agent_b):
    nc = get_nc()
    in_maps = prepare_in_maps(q, k, v, agent_w, agent_b)
    res = run_bass_kernel_spmd(nc, in_maps, list(range(N_CORES)))
    return gather(res.results)
